# revision 15
# baseline (speedup 1.0000x reference)
"""CrossTransFormer attention kernel for 8x Trainium2 NeuronCores (Bass/Tile).

Problem (per batch b, B=8, C=773, P=4096):
    K = Wk @ Xk + bk            [C, P]
    V = Wv @ Xq + bv            [C, P]
    S[i, j] = sum_c K[c, i] * V[c, j] / sqrt(C)       (i, j over P)
    H = softmax(S, axis=i)
    out[k, j] = sum_i Xk[k, i] * H[i, j]              [C, P]

Sharding: data-parallel over batch, one batch per NeuronCore, no collectives.

Algebraic refactor: softmax over the key axis i is invariant to adding a
per-column constant, and  S = K^T V = Xk^T (Wk^T Wv) Xq + Xk^T (Wk^T bv) 1^T
                             + 1 (bk^T V)           <- column-constant, drops.
So with  mT = (Wv^T Wk) = (Wk^T Wv)^T  and  m_v = Wk^T bv:
    V' = (Wk^T Wv) Xq + m_v 1^T        [C, P]   (single projection)
    S  = Xk^T V'                                 (Xk used directly as lhsT)
    out = Xk softmax_i(S / sqrt(C))
This removes the whole K-projection and all weight transposes (mT comes
straight from natural-layout Wk/Wv on the PE), and bk is never used.

Per-core algorithm (all matmul operands fp16, fp32 PSUM accumulation;
softmax without max-subtraction -- S ~ N(0,1) so exp is safe in fp32):
  A) mT[b, a] = sum_o Wv[o, b] Wk[o, a]  on the PE with bv appended as an
     extra lhsT column, so mT row 773 is m_v (pairs with a ones row in the
     streamed Xq chunks -- the bias add is free inside the projection).
  B/C) interleaved per 512-col chunk: stream Xk -> xk16 resident fp16
     [c, i] (S lhsT) + PE-transposed into qt = Xk^T fp16 [i, k] resident
     (out lhsT, with an appended ones column so the out matmul also
     computes softmax column sums for free); stream Xq -> project
     V' = mT.T @ Xq + m_v kept fully SBUF-resident fp16 (no DRAM staging).
  D) For each j-block of 512: loop i-tiles of 128: S-psum =
     xk16_tile.T @ v16_block (7 matmuls), exp via ACT with the 1/sqrt(C)
     scale fused, out_acc[k, j] += qt_tile.T @ expS (7 matmuls, held in 7
     PSUM banks across the whole i-loop; 8th bank ping-pongs S).
     Epilogue: spill acc banks to SBUF (frees them for the next j-block),
     then normalize by the softmax sums row and DMA out.

Hot-loop matmuls are all full 128-wide-stationary: the channel remainder
(c 768..772, 5 rows) is zero-padded in xk16/v16/mT, and the out k-tail
uses the overlapping stationary window qt[:, t, 704:832] whose ones
column lands the softmax sums on PSUM partition 96 (and k 768..772 on
partitions 64..68) -- sub-128 stationary tiles would run in PE
row/col-group mode, which costs ~2x190ns per inner step in mode switches.

DMA issue is split across both HWDGE queues (SP and ACT) -- a single
queue's ~1.1us per-dma_start issue+sem overhead otherwise caps input
streaming well below the 16 DMA engines' bandwidth.
"""

import sys

sys.path.insert(0, "/opt/trn_rl_repo")

import numpy as np

import concourse.bacc as bacc
import concourse.mybir as mybir
import concourse.tile as tile
from concourse.bass_utils import run_bass_kernel_spmd
from concourse.masks import make_identity

F32 = mybir.dt.float32
F16 = mybir.dt.float16

C = 773
PT = 128
CT = 7  # ceil(773 / 128) chunks of the channel dim
LC = C - (CT - 1) * PT  # 5 rows in the last chunk
JB = 512  # j-block width (one PSUM bank of fp32)
QW = 832  # qt row width: 773 ch + pad + ones col at 800 + pad to 832
KT6 = 704  # out k-tail stationary window start: qt[704:832]
SUMP = 800 - KT6  # softmax sums land on psum partition 96
K6P = 768 - KT6  # k=768..772 land on psum partitions 64..68


def build(P=4096, n_cores=8):
    NJ = P // JB
    IT = P // PT
    SCALE = float(1.0 / np.sqrt(C))

    nc = bacc.Bacc("TRN2", target_bir_lowering=False, debug=False,
                   num_devices=n_cores)
    Xq = nc.dram_tensor("Xq", [C, P], F32, kind="ExternalInput")
    Xk = nc.dram_tensor("Xk", [C, P], F32, kind="ExternalInput")
    Wk = nc.dram_tensor("Wk", [C, C], F32, kind="ExternalInput")
    bk = nc.dram_tensor("bk", [C], F32, kind="ExternalInput")  # unused (softmax shift)
    Wv = nc.dram_tensor("Wv", [C, C], F32, kind="ExternalInput")
    bv = nc.dram_tensor("bv", [C], F32, kind="ExternalInput")
    out = nc.dram_tensor("out", [C, P], F32, kind="ExternalOutput")

    with tile.TileContext(nc) as tc:
        with tc.tile_pool(name="persist", bufs=1) as persist:
            # warm tile memset FIRST in the gpsimd queue so the PE warmup
            # can start ~1us in (everything else on gpsimd queues behind it)
            warm = persist.tile([PT, JB], F16)
            nc.gpsimd.memset(warm[:, :], 0.0)
            ident = persist.tile([PT, PT], F16)
            make_identity(nc, ident)

            # projection lhsT: mT[b-in-tile, bt, a];  bt=6 rows: 0..4 = b
            # 768..772, row 5 = m_v (bias row, pairs with ones row in x16),
            # rows 6..127 zero so the projection runs full-128 stationary
            mT = persist.tile([PT, CT, 896], F16)
            # S lhsT: Xk in natural [c, i] layout, fp16; tile 6 rows 5..127
            # zeroed for full-128 S matmuls
            xk16 = persist.tile([PT, CT, P], F16)
            # out lhsT: [i-in-tile, it, k]; col 800 all-ones (softmax sums),
            # cols 773..799 and 801..831 zero
            qt = persist.tile([PT, IT, QW], F16)
            ones16 = persist.tile([1, JB], F16)
            nc.gpsimd.memset(ones16[:, :], 1.0)
            nc.gpsimd.memset(qt[:, :, C:], 0.0)
            nc.gpsimd.memset(qt[:, :, 800:801], 1.0)
            # full-tile zeroing (engine partition base must be 0/32/64/96);
            # real rows are written over the zeros later
            nc.gpsimd.memset(xk16[:, CT - 1, :], 0.0)
            nc.gpsimd.memset(mT[:, CT - 1, :], 0.0)
            nc.gpsimd.memset(mT[:, :, C:], 0.0)

            # prewarm the gpsimd partition_broadcast library (first use
            # otherwise pays an ~8us UNLOAD_LIB/LOAD_LIB at the first
            # j-block epilogue)
            wbc = persist.tile([PT, 16], F32)
            wbr = persist.tile([1, 16], F32)
            nc.gpsimd.memset(wbr[:, :], 1.0)
            nc.gpsimd.partition_broadcast(wbc[:], wbr[:])
            # PE warmup: dummy matmuls so the HAM clock-gate opens (4/8 ->
            # 8/8) and stays open while the weight DMAs land, and the exp
            # activation table loads before the main loop.
            with tc.tile_pool(name="pswarm", bufs=1, space="PSUM") as pswarm:
                wps = pswarm.tile([PT, JB], F32)
                for _ in range(72):
                    nc.tensor.matmul(wps[:, :], warm[:, :PT], warm[:, :],
                                     start=True, stop=True,
                                     skip_group_check=True)
                wexp = persist.tile([1, 16], F32)
                nc.scalar.activation(wexp[:], wps[:1, :16],
                                     mybir.ActivationFunctionType.Exp,
                                     scale=1.0)

            # ---- Phase A: mT = Wv^T Wk (+ bv bias column -> m_v row) ----
            with (
                tc.tile_pool(name="wload", bufs=5) as wload,
                tc.tile_pool(name="wres", bufs=1) as wres,
                tc.tile_pool(name="psA", bufs=2, space="PSUM") as psA,
            ):
                wk16 = wres.tile([PT, CT, C], F16)
                wv16 = wres.tile([PT, CT, C + 1], F16)  # col 773 = bv
                bvc = wres.tile([PT, CT, 1], F32)
                CopyA = mybir.ActivationFunctionType.Copy
                # one staging tile per (ot, matrix): all 14 W DMAs issue
                # immediately with no buffer-rotation waits
                wkf = [wload.tile([PT, C], F32, tag="wkf", name=f"wkf{ot}")
                       for ot in range(CT)]
                wvf = [wload.tile([PT, C], F32, tag="wvf", name=f"wvf{ot}")
                       for ot in range(CT)]
                for ot in range(CT):
                    po = PT if ot < CT - 1 else LC
                    nc.sync.dma_start(wkf[ot][:po, :],
                                      Wk[ot * PT:ot * PT + po, :])
                    nc.scalar.dma_start(wvf[ot][:po, :],
                                        Wv[ot * PT:ot * PT + po, :])
                for ot in range(CT):
                    po = PT if ot < CT - 1 else LC
                    nc.vector.tensor_copy(wk16[:po, ot, :], wkf[ot][:po, :])
                    nc.scalar.activation(wv16[:po, ot, :C], wvf[ot][:po, :],
                                         CopyA, scale=1.0)
                for ot in range(CT):
                    po = PT if ot < CT - 1 else LC
                    nc.scalar.dma_start(bvc[:po, ot, :],
                                        bv[ot * PT:ot * PT + po, None])
                    nc.vector.tensor_copy(wv16[:po, ot, C:C + 1],
                                          bvc[:po, ot, :])

                for bt in range(CT):
                    pb = PT if bt < CT - 1 else LC + 1  # +1: bias row
                    for asl, aw in ((slice(0, 512), 512), (slice(512, C), C - 512)):
                        ps = psA.tile([PT, 512], F32, tag="psA")
                        for ot in range(CT):
                            po = PT if ot < CT - 1 else LC
                            nc.tensor.matmul(
                                ps[:pb, :aw],
                                wv16[:po, ot, bt * PT:bt * PT + pb],
                                wk16[:po, ot, asl],
                                start=(ot == 0),
                                stop=(ot == CT - 1),
                            )
                        nc.any.tensor_copy(mT[:pb, bt, asl], ps[:pb, :aw])

            # ---- resident V' + phases B & C interleaved per 512 chunk ----
            with tc.tile_pool(name="resid", bufs=1) as resid:
                # V' projection, fully SBUF-resident; tile 6 rows 5..127
                # zeroed once for full-128 S matmuls
                v16 = resid.tile([PT, CT, P], F16)
                nc.gpsimd.memset(v16[:, CT - 1, :], 0.0)

                with (
                    tc.tile_pool(name="xs", bufs=3) as xs,
                    tc.tile_pool(name="xq", bufs=2) as xq,
                    tc.tile_pool(name="psP", bufs=2, space="PSUM") as psP,
                    tc.tile_pool(name="pst", bufs=4, space="PSUM") as pst,
                ):
                    Copy = mybir.ActivationFunctionType.Copy
                    for jc in range(NJ):
                        js = slice(jc * JB, (jc + 1) * JB)
                        # all DMA issues upfront (Xk on the SP queue, Xq on
                        # the ACT queue) so per-queue issue never blocks
                        # behind a cast waiting for data
                        xkf, xqf = [], []
                        for ct in range(CT):
                            pc = PT if ct < CT - 1 else LC
                            xf = xs.tile([PT, JB], F32, tag="xkf")
                            nc.sync.dma_start(
                                xf[:pc, :], Xk[ct * PT:ct * PT + pc, js])
                            xkf.append(xf)
                        x16 = xq.tile([PT, CT, JB], F16, tag="x16")
                        nc.gpsimd.memset(x16[:, CT - 1, :], 0.0)
                        for ct in range(CT):
                            pc = PT if ct < CT - 1 else LC
                            xf = xs.tile([PT, JB], F32, tag="xqf")
                            nc.scalar.dma_start(
                                xf[:pc, :], Xq[ct * PT:ct * PT + pc, js])
                            xqf.append(xf)
                        # casts: Xk on DVE, Xq on ACT (Copy activation)
                        for ct in range(CT):
                            pc = PT if ct < CT - 1 else LC
                            nc.vector.tensor_copy(xk16[:pc, ct, js],
                                                  xkf[ct][:pc, :])
                        for ct in range(CT):
                            pc = PT if ct < CT - 1 else LC
                            nc.scalar.activation(x16[:pc, ct, :],
                                                 xqf[ct][:pc, :],
                                                 Copy, scale=1.0)
                        nc.scalar.dma_start(x16[LC:LC + 1, CT - 1, :],
                                            ones16[:, :])
                        # PE transposes -> one batched psum row -> qt
                        for sub in range(JB // PT):
                            it = jc * (JB // PT) + sub
                            isl = slice(jc * JB + sub * PT,
                                        jc * JB + (sub + 1) * PT)
                            ps = pst.tile([PT, C], F16, tag="pst")
                            for kt in range(CT):
                                pk = PT if kt < CT - 1 else LC
                                nc.tensor.transpose(
                                    ps[:, kt * PT:kt * PT + pk],
                                    xk16[:pk, kt, isl],
                                    ident[:pk, :pk],
                                )
                            nc.vector.tensor_copy(qt[:, it, :C], ps[:, :])
                        # projection: V'[:, js] resident in v16
                        for ot in range(CT):
                            po = PT if ot < CT - 1 else LC
                            ps = psP.tile([PT, JB], F32, tag="psP")
                            for ct in range(CT):
                                nc.tensor.matmul(
                                    ps[:, :],
                                    mT[:, ct, ot * PT:(ot + 1) * PT],
                                    x16[:, ct, :],
                                    start=(ct == 0),
                                    stop=(ct == CT - 1),
                                )
                            nc.vector.tensor_copy(v16[:po, ot, js],
                                                  ps[:po, :])

                # ---- Phase D: attention main loop ----
                # Flat software-pipelined schedule over global steps
                # g = jb*IT + t.  Per step (all on the PE, in order):
                #   [out(g-1) kt=5,6] [epilogue(jb-1) if t==0] [S(g+1)]
                #   [exp(g) on ACT] [out(g) kt=0..4]
                # The two out-tail matmuls between S(g)'s stop and S(g+1)'s
                # start give the ACT exp(g) time to drain the single S PSUM
                # bank (no spare bank to double-buffer S: 7 acc + 1 S = 8).
                # S of each j-block's step 0 is emitted in the previous
                # block's empty S-slot so its exp is also covered.
                with (
                    tc.tile_pool(name="ep", bufs=3) as epl,
                    tc.tile_pool(name="sb", bufs=1) as sbp,
                    tc.tile_pool(name="rp", bufs=2) as rp,
                    tc.tile_pool(name="psacc", bufs=CT, space="PSUM") as psacc,
                    tc.tile_pool(name="pss", bufs=1, space="PSUM") as pss,
                ):
                    sb_acc = sbp.tile([PT, CT, JB], F32)
                    TOT = NJ * IT

                    def emit_S(g):
                        jb, t = divmod(g, IT)
                        ps_s = pss.tile([PT, JB], F32, tag="s")
                        tsl = slice(t * PT, (t + 1) * PT)
                        js = slice(jb * JB, (jb + 1) * JB)
                        for ct in range(CT):
                            nc.tensor.matmul(
                                ps_s[:, :],
                                xk16[:, ct, tsl],
                                v16[:, ct, js],
                                start=(ct == 0),
                                stop=(ct == CT - 1),
                                skip_group_check=True,
                            )
                        return ps_s

                    def emit_out(acc, g, es, kts):
                        t = g % IT
                        for kt in kts:
                            ksl = (slice(kt * PT, (kt + 1) * PT)
                                   if kt < CT - 1 else slice(KT6, QW))
                            nc.tensor.matmul(
                                acc[kt][:, :],
                                qt[:, t, ksl],
                                es[:],
                                start=(t == 0),
                                stop=(t == IT - 1),
                                skip_group_check=True,
                            )

                    def emit_epilogue(jb, acc, last):
                        js = slice(jb * JB, (jb + 1) * JB)
                        recip = rp.tile([1, JB], F32, tag="recip")
                        bcst = rp.tile([PT, JB], F32, tag="bc")
                        if not last:
                            # spill acc banks to SBUF first so the next
                            # j-block's out matmuls can reuse them at once
                            for kt in range(CT - 1):
                                nc.vector.tensor_copy(sb_acc[:, kt, :],
                                                      acc[kt][:, :])
                            nc.vector.tensor_copy(
                                sb_acc[K6P:SUMP + 1, CT - 1, :],
                                acc[CT - 1][K6P:SUMP + 1, :])
                            nc.vector.reciprocal(
                                recip[:], sb_acc[SUMP:SUMP + 1, CT - 1, :])
                            nc.gpsimd.partition_broadcast(bcst[:], recip[:])
                            for kt in range(CT - 1):
                                nc.vector.tensor_mul(
                                    out=sb_acc[:, kt, :],
                                    in0=sb_acc[:, kt, :], in1=bcst[:, :])
                                nc.sync.dma_start(
                                    out[kt * PT:(kt + 1) * PT, js],
                                    sb_acc[:, kt, :])
                            nc.vector.tensor_mul(
                                out=sb_acc[K6P:K6P + LC, CT - 1, :],
                                in0=sb_acc[K6P:K6P + LC, CT - 1, :],
                                in1=bcst[K6P:K6P + LC, :])
                            nc.sync.dma_start(
                                out[(CT - 1) * PT:C, js],
                                sb_acc[K6P:K6P + LC, CT - 1, :])
                        else:
                            # final j-block: normalize straight from PSUM
                            # (nothing reuses the banks; shortens the tail)
                            nc.vector.reciprocal(
                                recip[:], acc[CT - 1][SUMP:SUMP + 1, :])
                            nc.gpsimd.partition_broadcast(bcst[:], recip[:])
                            for kt in range(CT - 1):
                                nc.vector.tensor_mul(
                                    out=sb_acc[:, kt, :],
                                    in0=acc[kt][:, :], in1=bcst[:, :])
                                nc.sync.dma_start(
                                    out[kt * PT:(kt + 1) * PT, js],
                                    sb_acc[:, kt, :])
                            nc.vector.tensor_mul(
                                out=sb_acc[K6P:K6P + LC, CT - 1, :],
                                in0=acc[CT - 1][K6P:K6P + LC, :],
                                in1=bcst[K6P:K6P + LC, :])
                            nc.sync.dma_start(
                                out[(CT - 1) * PT:C, js],
                                sb_acc[K6P:K6P + LC, CT - 1, :])

                    acc_cur = None
                    acc_prev = None
                    es_prev = None
                    ps_cur = None
                    for g in range(TOT):
                        jb, t = divmod(g, IT)
                        if t == 0:
                            acc_prev = acc_cur
                            acc_cur = [psacc.tile([PT, JB], F32, tag="acc",
                                                  name=f"acc{jb}_{i}")
                                       for i in range(CT)]
                        if g == 0:
                            ps_cur = emit_S(0)
                        if g > 0:
                            # out-tail of the previous step (possibly the
                            # previous j-block's final step)
                            emit_out(acc_prev if t == 0 else acc_cur,
                                     g - 1, es_prev, range(5, CT))
                        if t == 0 and jb > 0:
                            emit_epilogue(jb - 1, acc_prev, last=False)
                        if g < TOT - 1:
                            ps_next = emit_S(g + 1)
                        else:
                            ps_next = None
                        es = epl.tile([PT, JB], F16, tag="es")
                        nc.scalar.activation(
                            es[:], ps_cur[:],
                            mybir.ActivationFunctionType.Exp, scale=SCALE,
                        )
                        emit_out(acc_cur, g, es, range(5))
                        ps_cur, es_prev = ps_next, es
                    emit_out(acc_cur, TOT - 1, es_prev, range(5, CT))
                    emit_epilogue(NJ - 1, acc_cur, last=True)

    nc.compile()
    return nc


_CACHE = {}


def _get_program(P=4096, n_cores=8):
    key = (P, n_cores)
    if key not in _CACHE:
        _CACHE[key] = build(P, n_cores)
    return _CACHE[key]


def _run(inputs, trace=False, **kw):
    nc = _get_program()
    Xq = np.asarray(inputs["Xq"], dtype=np.float32)
    Xk = np.asarray(inputs["Xk"], dtype=np.float32)
    Wk = np.ascontiguousarray(np.asarray(inputs["Wk"], dtype=np.float32))
    bkv = np.ascontiguousarray(np.asarray(inputs["bk"], dtype=np.float32))
    Wv = np.ascontiguousarray(np.asarray(inputs["Wv"], dtype=np.float32))
    bvv = np.ascontiguousarray(np.asarray(inputs["bv"], dtype=np.float32))
    B = Xq.shape[0]
    in_maps = [
        {
            "Xq": np.ascontiguousarray(Xq[b]),
            "Xk": np.ascontiguousarray(Xk[b]),
            "Wk": Wk,
            "bk": bkv,
            "Wv": Wv,
            "bv": bvv,
        }
        for b in range(B)
    ]
    res = run_bass_kernel_spmd(nc, in_maps, list(range(B)), trace=trace, **kw)
    outs = np.stack([res.results[b]["out"] for b in range(B)], axis=0)
    return outs.astype(np.float32), res


def kernel(**inputs):
    outs, _ = _run(inputs)
    return outs


# revision 18
# speedup vs baseline: 1.0045x; 1.0045x over previous
"""CrossTransFormer attention kernel for 8x Trainium2 NeuronCores (Bass/Tile).

Problem (per batch b, B=8, C=773, P=4096):
    K = Wk @ Xk + bk            [C, P]
    V = Wv @ Xq + bv            [C, P]
    S[i, j] = sum_c K[c, i] * V[c, j] / sqrt(C)       (i, j over P)
    H = softmax(S, axis=i)
    out[k, j] = sum_i Xk[k, i] * H[i, j]              [C, P]

Sharding: data-parallel over batch, one batch per NeuronCore, no collectives.

Algebraic refactor: softmax over the key axis i is invariant to adding a
per-column constant, and  S = K^T V = Xk^T (Wk^T Wv) Xq + Xk^T (Wk^T bv) 1^T
                             + 1 (bk^T V)           <- column-constant, drops.
So with  mT = (Wv^T Wk) = (Wk^T Wv)^T  and  m_v = Wk^T bv:
    V' = (Wk^T Wv) Xq + m_v 1^T        [C, P]   (single projection)
    S  = Xk^T V'                                 (Xk used directly as lhsT)
    out = Xk softmax_i(S / sqrt(C))
This removes the whole K-projection and all weight transposes (mT comes
straight from natural-layout Wk/Wv on the PE), and bk is never used.

Per-core algorithm (all matmul operands fp16, fp32 PSUM accumulation;
softmax without max-subtraction -- S ~ N(0,1) so exp is safe in fp32):
  A) mT[b, a] = sum_o Wv[o, b] Wk[o, a]  on the PE with bv appended as an
     extra lhsT column, so mT row 773 is m_v (pairs with a ones row in the
     streamed Xq chunks -- the bias add is free inside the projection).
  B/C) interleaved per 512-col chunk: stream Xk -> xk16 resident fp16
     [c, i] (S lhsT) + PE-transposed into qt = Xk^T fp16 [i, k] resident
     (out lhsT, with an appended ones column so the out matmul also
     computes softmax column sums for free); stream Xq -> project
     V' = mT.T @ Xq + m_v kept fully SBUF-resident fp16 (no DRAM staging).
  D) For each j-block of 512: loop i-tiles of 128: S-psum =
     xk16_tile.T @ v16_block (7 matmuls), exp via ACT with the 1/sqrt(C)
     scale fused, out_acc[k, j] += qt_tile.T @ expS (7 matmuls, held in 7
     PSUM banks across the whole i-loop; 8th bank ping-pongs S).
     Epilogue: spill acc banks to SBUF (frees them for the next j-block),
     then normalize by the softmax sums row and DMA out.

Hot-loop matmuls are all full 128-wide-stationary: the channel remainder
(c 768..772, 5 rows) is zero-padded in xk16/v16/mT, and the out k-tail
uses the overlapping stationary window qt[:, t, 704:832] whose ones
column lands the softmax sums on PSUM partition 96 (and k 768..772 on
partitions 64..68) -- sub-128 stationary tiles would run in PE
row/col-group mode, which costs ~2x190ns per inner step in mode switches.

DMA issue is split across both HWDGE queues (SP and ACT) -- a single
queue's ~1.1us per-dma_start issue+sem overhead otherwise caps input
streaming well below the 16 DMA engines' bandwidth.
"""

import sys

sys.path.insert(0, "/opt/trn_rl_repo")

import numpy as np

import concourse.bacc as bacc
import concourse.mybir as mybir
import concourse.tile as tile
from concourse.bass_utils import run_bass_kernel_spmd
from concourse.masks import make_identity

F32 = mybir.dt.float32
F16 = mybir.dt.float16

C = 773
PT = 128
CT = 7  # ceil(773 / 128) chunks of the channel dim
LC = C - (CT - 1) * PT  # 5 rows in the last chunk
JB = 512  # j-block width (one PSUM bank of fp32)
QW = 832  # qt row width: 773 ch + pad + ones col at 800 + pad to 832
KT6 = 704  # out k-tail stationary window start: qt[704:832]
SUMP = 800 - KT6  # softmax sums land on psum partition 96
K6P = 768 - KT6  # k=768..772 land on psum partitions 64..68


def build(P=4096, n_cores=8):
    NJ = P // JB
    IT = P // PT
    SCALE = float(1.0 / np.sqrt(C))

    nc = bacc.Bacc("TRN2", target_bir_lowering=False, debug=False,
                   num_devices=n_cores)
    Xq = nc.dram_tensor("Xq", [C, P], F32, kind="ExternalInput")
    Xk = nc.dram_tensor("Xk", [C, P], F32, kind="ExternalInput")
    Wk = nc.dram_tensor("Wk", [C, C], F32, kind="ExternalInput")
    bk = nc.dram_tensor("bk", [C], F32, kind="ExternalInput")  # unused (softmax shift)
    Wv = nc.dram_tensor("Wv", [C, C], F32, kind="ExternalInput")
    bv = nc.dram_tensor("bv", [C], F32, kind="ExternalInput")
    out = nc.dram_tensor("out", [C, P], F32, kind="ExternalOutput")

    with tile.TileContext(nc) as tc:
        with tc.tile_pool(name="persist", bufs=1) as persist:
            # warm tile memset FIRST in the gpsimd queue so the PE warmup
            # can start ~1us in (everything else on gpsimd queues behind it)
            warm = persist.tile([PT, JB], F16)
            nc.vector.memset(warm[:, :], 0.0)
            ident = persist.tile([PT, PT], F16)
            make_identity(nc, ident)

            # projection lhsT: mT[b-in-tile, bt, a];  bt=6 rows: 0..4 = b
            # 768..772, row 5 = m_v (bias row, pairs with ones row in x16),
            # rows 6..127 zero so the projection runs full-128 stationary
            mT = persist.tile([PT, CT, 896], F16)
            # S lhsT: Xk in natural [c, i] layout, fp16; tile 6 rows 5..127
            # zeroed for full-128 S matmuls
            xk16 = persist.tile([PT, CT, P], F16)
            # out lhsT: [i-in-tile, it, k]; col 800 all-ones (softmax sums),
            # cols 773..799 and 801..831 zero
            qt = persist.tile([PT, IT, QW], F16)
            ones16 = persist.tile([1, JB], F16)
            nc.gpsimd.memset(ones16[:, :], 1.0)
            nc.gpsimd.memset(qt[:, :, C:], 0.0)
            nc.gpsimd.memset(qt[:, :, 800:801], 1.0)
            # full-tile zeroing (engine partition base must be 0/32/64/96);
            # real rows are written over the zeros later
            nc.gpsimd.memset(xk16[:, CT - 1, :], 0.0)
            nc.gpsimd.memset(mT[:, CT - 1, :], 0.0)
            nc.gpsimd.memset(mT[:, :, C:], 0.0)

            # prewarm the gpsimd partition_broadcast library (first use
            # otherwise pays an ~8us UNLOAD_LIB/LOAD_LIB at the first
            # j-block epilogue)
            wbc = persist.tile([PT, 16], F32)
            wbr = persist.tile([1, 16], F32)
            nc.gpsimd.memset(wbr[:, :], 1.0)
            nc.gpsimd.partition_broadcast(wbc[:], wbr[:])
            # PE warmup: dummy matmuls so the HAM clock-gate opens (4/8 ->
            # 8/8) and stays open while the weight DMAs land, and the exp
            # activation table loads before the main loop.
            with tc.tile_pool(name="pswarm", bufs=1, space="PSUM") as pswarm:
                wps = pswarm.tile([PT, JB], F32)
                for _ in range(72):
                    nc.tensor.matmul(wps[:, :], warm[:, :PT], warm[:, :],
                                     start=True, stop=True,
                                     skip_group_check=True)
                wexp = persist.tile([1, 16], F32)
                nc.scalar.activation(wexp[:], wps[:1, :16],
                                     mybir.ActivationFunctionType.Exp,
                                     scale=1.0)

            # ---- Phase A: mT = Wv^T Wk (+ bv bias column -> m_v row) ----
            # Wk arrives as one big 3D DMA (+5-row remainder); Wv through a
            # 3-deep rotation on the ACT queue; all casts on DVE
            with (
                tc.tile_pool(name="wkbig", bufs=1) as wkbig,
                tc.tile_pool(name="wload", bufs=3) as wload,
                tc.tile_pool(name="wres", bufs=1) as wres,
                tc.tile_pool(name="psA", bufs=2, space="PSUM") as psA,
            ):
                wk16 = wres.tile([PT, CT, C], F16)
                wv16 = wres.tile([PT, CT, C + 1], F16)  # col 773 = bv
                bvc = wres.tile([PT, CT, 1], F32)
                wkf32 = wkbig.tile([PT, CT, C], F32)
                nc.sync.dma_start(
                    wkf32[:, :CT - 1, :],
                    Wk[:(CT - 1) * PT, :].rearrange("(ct p) c -> p ct c",
                                                    p=PT))
                nc.sync.dma_start(wkf32[:LC, CT - 1, :],
                                  Wk[(CT - 1) * PT:, :])
                wvf = [wload.tile([PT, C], F32, tag="wvf", name=f"wvf{ot}")
                       for ot in range(CT)]
                for ot in range(CT):
                    po = PT if ot < CT - 1 else LC
                    nc.scalar.dma_start(wvf[ot][:po, :],
                                        Wv[ot * PT:ot * PT + po, :])
                for ot in range(CT):
                    po = PT if ot < CT - 1 else LC
                    nc.vector.tensor_copy(wk16[:po, ot, :],
                                          wkf32[:po, ot, :])
                    nc.vector.tensor_copy(wv16[:po, ot, :C], wvf[ot][:po, :])
                for ot in range(CT):
                    po = PT if ot < CT - 1 else LC
                    nc.scalar.dma_start(bvc[:po, ot, :],
                                        bv[ot * PT:ot * PT + po, None])
                    nc.vector.tensor_copy(wv16[:po, ot, C:C + 1],
                                          bvc[:po, ot, :])

                for bt in range(CT):
                    pb = PT if bt < CT - 1 else LC + 1  # +1: bias row
                    for asl, aw in ((slice(0, 512), 512), (slice(512, C), C - 512)):
                        ps = psA.tile([PT, 512], F32, tag="psA")
                        for ot in range(CT):
                            po = PT if ot < CT - 1 else LC
                            nc.tensor.matmul(
                                ps[:pb, :aw],
                                wv16[:po, ot, bt * PT:bt * PT + pb],
                                wk16[:po, ot, asl],
                                start=(ot == 0),
                                stop=(ot == CT - 1),
                            )
                        nc.any.tensor_copy(mT[:pb, bt, asl], ps[:pb, :aw])

            # ---- resident V' + phases B & C interleaved per 512 chunk ----
            with tc.tile_pool(name="resid", bufs=1) as resid:
                # V' projection, fully SBUF-resident; tile 6 rows 5..127
                # zeroed once for full-128 S matmuls
                v16 = resid.tile([PT, CT, P], F16)
                nc.gpsimd.memset(v16[:, CT - 1, :], 0.0)

                with (
                    tc.tile_pool(name="xs", bufs=3) as xs,
                    tc.tile_pool(name="xq", bufs=2) as xq,
                    tc.tile_pool(name="psP", bufs=2, space="PSUM") as psP,
                    tc.tile_pool(name="pst", bufs=4, space="PSUM") as pst,
                ):
                    Copy = mybir.ActivationFunctionType.Copy
                    for jc in range(NJ):
                        js = slice(jc * JB, (jc + 1) * JB)
                        # all DMA issues upfront (Xk on the SP queue, Xq on
                        # the ACT queue) so per-queue issue never blocks
                        # behind a cast waiting for data
                        xkf, xqf = [], []
                        for ct in range(CT):
                            pc = PT if ct < CT - 1 else LC
                            xf = xs.tile([PT, JB], F32, tag="xkf")
                            nc.sync.dma_start(
                                xf[:pc, :], Xk[ct * PT:ct * PT + pc, js])
                            xkf.append(xf)
                        x16 = xq.tile([PT, CT, JB], F16, tag="x16")
                        nc.gpsimd.memset(x16[:, CT - 1, :], 0.0)
                        for ct in range(CT):
                            pc = PT if ct < CT - 1 else LC
                            xf = xs.tile([PT, JB], F32, tag="xqf")
                            nc.scalar.dma_start(
                                xf[:pc, :], Xq[ct * PT:ct * PT + pc, js])
                            xqf.append(xf)
                        # casts: Xk on DVE, Xq on ACT (Copy activation)
                        for ct in range(CT):
                            pc = PT if ct < CT - 1 else LC
                            nc.vector.tensor_copy(xk16[:pc, ct, js],
                                                  xkf[ct][:pc, :])
                        for ct in range(CT):
                            pc = PT if ct < CT - 1 else LC
                            nc.scalar.activation(x16[:pc, ct, :],
                                                 xqf[ct][:pc, :],
                                                 Copy, scale=1.0)
                        nc.scalar.dma_start(x16[LC:LC + 1, CT - 1, :],
                                            ones16[:, :])
                        # PE transposes -> one batched psum row -> qt
                        for sub in range(JB // PT):
                            it = jc * (JB // PT) + sub
                            isl = slice(jc * JB + sub * PT,
                                        jc * JB + (sub + 1) * PT)
                            ps = pst.tile([PT, C], F16, tag="pst")
                            for kt in range(CT):
                                pk = PT if kt < CT - 1 else LC
                                nc.tensor.transpose(
                                    ps[:, kt * PT:kt * PT + pk],
                                    xk16[:pk, kt, isl],
                                    ident[:pk, :pk],
                                )
                            nc.vector.tensor_copy(qt[:, it, :C], ps[:, :])
                        # projection: V'[:, js] resident in v16
                        for ot in range(CT):
                            po = PT if ot < CT - 1 else LC
                            ps = psP.tile([PT, JB], F32, tag="psP")
                            for ct in range(CT):
                                nc.tensor.matmul(
                                    ps[:, :],
                                    mT[:, ct, ot * PT:(ot + 1) * PT],
                                    x16[:, ct, :],
                                    start=(ct == 0),
                                    stop=(ct == CT - 1),
                                )
                            nc.vector.tensor_copy(v16[:po, ot, js],
                                                  ps[:po, :])

                # ---- Phase D: attention main loop ----
                # Flat software-pipelined schedule over global steps
                # g = jb*IT + t.  Per step (all on the PE, in order):
                #   [out(g-1) kt=5,6] [epilogue(jb-1) if t==0] [S(g+1)]
                #   [exp(g) on ACT] [out(g) kt=0..4]
                # The two out-tail matmuls between S(g)'s stop and S(g+1)'s
                # start give the ACT exp(g) time to drain the single S PSUM
                # bank (no spare bank to double-buffer S: 7 acc + 1 S = 8).
                # S of each j-block's step 0 is emitted in the previous
                # block's empty S-slot so its exp is also covered.
                with (
                    tc.tile_pool(name="ep", bufs=3) as epl,
                    tc.tile_pool(name="sb", bufs=1) as sbp,
                    tc.tile_pool(name="rp", bufs=2) as rp,
                    tc.tile_pool(name="psacc", bufs=CT, space="PSUM") as psacc,
                    tc.tile_pool(name="pss", bufs=1, space="PSUM") as pss,
                ):
                    sb_acc = sbp.tile([PT, CT, JB], F32)
                    TOT = NJ * IT

                    def emit_S(g):
                        jb, t = divmod(g, IT)
                        ps_s = pss.tile([PT, JB], F32, tag="s")
                        tsl = slice(t * PT, (t + 1) * PT)
                        js = slice(jb * JB, (jb + 1) * JB)
                        for ct in range(CT):
                            nc.tensor.matmul(
                                ps_s[:, :],
                                xk16[:, ct, tsl],
                                v16[:, ct, js],
                                start=(ct == 0),
                                stop=(ct == CT - 1),
                                skip_group_check=True,
                            )
                        return ps_s

                    def emit_out(acc, g, es, kts):
                        t = g % IT
                        for kt in kts:
                            ksl = (slice(kt * PT, (kt + 1) * PT)
                                   if kt < CT - 1 else slice(KT6, QW))
                            nc.tensor.matmul(
                                acc[kt][:, :],
                                qt[:, t, ksl],
                                es[:],
                                start=(t == 0),
                                stop=(t == IT - 1),
                                skip_group_check=True,
                            )

                    def emit_epilogue(jb, acc, last):
                        js = slice(jb * JB, (jb + 1) * JB)
                        recip = rp.tile([1, JB], F32, tag="recip")
                        bcst = rp.tile([PT, JB], F32, tag="bc")
                        if not last:
                            # spill acc banks to SBUF first so the next
                            # j-block's out matmuls can reuse them at once
                            for kt in range(CT - 1):
                                nc.vector.tensor_copy(sb_acc[:, kt, :],
                                                      acc[kt][:, :])
                            nc.vector.tensor_copy(
                                sb_acc[K6P:SUMP + 1, CT - 1, :],
                                acc[CT - 1][K6P:SUMP + 1, :])
                            nc.vector.reciprocal(
                                recip[:], sb_acc[SUMP:SUMP + 1, CT - 1, :])
                            nc.gpsimd.partition_broadcast(bcst[:], recip[:])
                            for kt in range(CT - 1):
                                nc.vector.tensor_mul(
                                    out=sb_acc[:, kt, :],
                                    in0=sb_acc[:, kt, :], in1=bcst[:, :])
                                nc.sync.dma_start(
                                    out[kt * PT:(kt + 1) * PT, js],
                                    sb_acc[:, kt, :])
                            nc.vector.tensor_mul(
                                out=sb_acc[K6P:K6P + LC, CT - 1, :],
                                in0=sb_acc[K6P:K6P + LC, CT - 1, :],
                                in1=bcst[K6P:K6P + LC, :])
                            nc.sync.dma_start(
                                out[(CT - 1) * PT:C, js],
                                sb_acc[K6P:K6P + LC, CT - 1, :])
                        else:
                            # final j-block: normalize straight from PSUM
                            # (nothing reuses the banks; shortens the tail)
                            nc.vector.reciprocal(
                                recip[:], acc[CT - 1][SUMP:SUMP + 1, :])
                            nc.gpsimd.partition_broadcast(bcst[:], recip[:])
                            for kt in range(CT - 1):
                                nc.vector.tensor_mul(
                                    out=sb_acc[:, kt, :],
                                    in0=acc[kt][:, :], in1=bcst[:, :])
                                nc.sync.dma_start(
                                    out[kt * PT:(kt + 1) * PT, js],
                                    sb_acc[:, kt, :])
                            nc.vector.tensor_mul(
                                out=sb_acc[K6P:K6P + LC, CT - 1, :],
                                in0=acc[CT - 1][K6P:K6P + LC, :],
                                in1=bcst[K6P:K6P + LC, :])
                            nc.sync.dma_start(
                                out[(CT - 1) * PT:C, js],
                                sb_acc[K6P:K6P + LC, CT - 1, :])

                    acc_cur = None
                    acc_prev = None
                    es_prev = None
                    ps_cur = None
                    for g in range(TOT):
                        jb, t = divmod(g, IT)
                        if t == 0:
                            acc_prev = acc_cur
                            acc_cur = [psacc.tile([PT, JB], F32, tag="acc",
                                                  name=f"acc{jb}_{i}")
                                       for i in range(CT)]
                        if g == 0:
                            ps_cur = emit_S(0)
                        if g > 0:
                            # out-tail of the previous step (possibly the
                            # previous j-block's final step)
                            emit_out(acc_prev if t == 0 else acc_cur,
                                     g - 1, es_prev, range(5, CT))
                        if t == 0 and jb > 0:
                            emit_epilogue(jb - 1, acc_prev, last=False)
                        if g < TOT - 1:
                            ps_next = emit_S(g + 1)
                        else:
                            ps_next = None
                        es = epl.tile([PT, JB], F16, tag="es")
                        nc.scalar.activation(
                            es[:], ps_cur[:],
                            mybir.ActivationFunctionType.Exp, scale=SCALE,
                        )
                        emit_out(acc_cur, g, es, range(5))
                        ps_cur, es_prev = ps_next, es
                    emit_out(acc_cur, TOT - 1, es_prev, range(5, CT))
                    emit_epilogue(NJ - 1, acc_cur, last=True)

    nc.compile()
    return nc


_CACHE = {}


def _get_program(P=4096, n_cores=8):
    key = (P, n_cores)
    if key not in _CACHE:
        _CACHE[key] = build(P, n_cores)
    return _CACHE[key]


def _run(inputs, trace=False, **kw):
    nc = _get_program()
    Xq = np.asarray(inputs["Xq"], dtype=np.float32)
    Xk = np.asarray(inputs["Xk"], dtype=np.float32)
    Wk = np.ascontiguousarray(np.asarray(inputs["Wk"], dtype=np.float32))
    bkv = np.ascontiguousarray(np.asarray(inputs["bk"], dtype=np.float32))
    Wv = np.ascontiguousarray(np.asarray(inputs["Wv"], dtype=np.float32))
    bvv = np.ascontiguousarray(np.asarray(inputs["bv"], dtype=np.float32))
    B = Xq.shape[0]
    in_maps = [
        {
            "Xq": np.ascontiguousarray(Xq[b]),
            "Xk": np.ascontiguousarray(Xk[b]),
            "Wk": Wk,
            "bk": bkv,
            "Wv": Wv,
            "bv": bvv,
        }
        for b in range(B)
    ]
    res = run_bass_kernel_spmd(nc, in_maps, list(range(B)), trace=trace, **kw)
    outs = np.stack([res.results[b]["out"] for b in range(B)], axis=0)
    return outs.astype(np.float32), res


def kernel(**inputs):
    outs, _ = _run(inputs)
    return outs
